# revision 1
# baseline (speedup 1.0000x reference)
"""Trainium2 Bass kernel for nn_DecoderBlock (self-attn + cross-attn + MLP).

Sharding: pure data-parallel over batch (B=8 -> 8 NeuronCores), no
collectives. Per core the whole block runs feature-major (features on SBUF
partitions, tokens on the free axis). LayerNorm gamma/beta are folded into
the following weights (host). RoPE rotate-half runs as one 128x128 +/-1
permutation matmul per head-pair block (instead of doubled Q/K projections);
the combine q*cos + rot(q)*sin is on DVE with bf16 cos/sin tables. Softmax
is unnormalized exp (ScalarE, 1/sqrt(d) folded into the exp scale); the
denominator comes from an all-ones column packed into the V lhsT, is
broadcast across partitions with a tiny [2,128] selector matmul, and the
normalization is a DVE divide (no Ln/Exp reciprocal, no DMA bounce).
Cross-attention K/V prep (y LayerNorm, V build, K rope) is interleaved into
the self-attention loop to fill tensor-engine stalls; LN statistics for the
post-attention norms accumulate inside the preceding projection loops.
Matmuls run in bf16 (fp32 PSUM accumulation); the residual stays fp32.
"""
import numpy as np

DIM, HEADS, HD = 768, 12, 64
N = 1024
B = 8
EPS = 1e-5
FREQ = 100.0
P = 128
C = DIM // P            # 6 feature chunks
NP = HEADS // 2         # 6 head pairs
KC = N // P             # 8 key chunks
T = N // P              # 8 token chunks
HM = (4 * DIM) // P     # 24 hidden chunks
HH = HM // 2            # 12 hidden chunks per MLP half


# ---------------------------------------------------------------- host prep

def _rope_tables(pos2d):
    """pos2d [N,2] int -> cos, sin [64, N] fp32 (y-half then x-half)."""
    j = np.arange(16, dtype=np.float32)
    inv = 1.0 / (FREQ ** (2.0 * j / 32.0))
    n = pos2d.shape[0]
    c = np.empty((64, n), np.float32)
    s = np.empty((64, n), np.float32)
    for half, p in ((0, pos2d[:, 0]), (1, pos2d[:, 1])):
        f = p.astype(np.float32)[None, :] * inv[:, None]
        emb = np.concatenate([f, f], 0)
        c[half * 32:(half + 1) * 32] = np.cos(emb)
        s[half * 32:(half + 1) * 32] = np.sin(emb)
    return c, s


def _feat_major(b):
    return np.ascontiguousarray(b.reshape(-1, P).T.astype(np.float32))


def _rot_mat():
    """RT[k, p]: out[p] = -in[p+16] (p%32<16), +in[p-16] (p%32>=16)."""
    r = np.zeros((P, P), np.float32)
    for base in range(0, P, 32):
        for i in range(16):
            r[base + i + 16, base + i] = -1.0
            r[base + i, base + i + 16] = 1.0
    return r


def prep_host(inputs):
    """Returns a list of per-core input dicts (weights shared)."""
    import ml_dtypes
    f32 = np.float32
    bf = ml_dtypes.bfloat16
    inp = {k: np.asarray(v) for k, v in inputs.items()}
    g1, b1 = inp['norm1_g'].astype(f32), inp['norm1_b'].astype(f32)
    g2, b2 = inp['norm2_g'].astype(f32), inp['norm2_b'].astype(f32)
    g3, b3 = inp['norm3_g'].astype(f32), inp['norm3_b'].astype(f32)
    gy, by = inp['normy_g'].astype(f32), inp['normy_b'].astype(f32)
    qkv = inp['qkv_w'].astype(f32)
    wq, wk, wv = qkv[:, :DIM], qkv[:, DIM:2 * DIM], qkv[:, 2 * DIM:]

    def fold(g, b, wmat):
        return (g[:, None] * wmat).astype(f32), (b @ wmat).astype(f32)

    wqA, bq = fold(g1, b1, wq)
    wkA, bk = fold(g1, b1, wk)
    wvF, bv = fold(g1, b1, wv)
    wcqA, bcq = fold(g2, b2, inp['projq_w'].astype(f32))
    wckA, bck = fold(gy, by, inp['projk_w'].astype(f32))
    wcvF, bcv = fold(gy, by, inp['projv_w'].astype(f32))
    wfc1, bfc1x = fold(g3, b3, inp['fc1_w'].astype(f32))
    bfc1 = inp['fc1_b'].astype(f32) + bfc1x

    def tiled(w):
        rows, cols = w.shape
        cr, ncb = rows // P, cols // P
        return np.ascontiguousarray(
            np.transpose(w.reshape(cr, P, ncb, P), (2, 1, 0, 3)).astype(bf))

    def vfull(w):
        cr = w.shape[0] // P
        return np.ascontiguousarray(
            np.transpose(w.reshape(cr, P, w.shape[1]), (1, 0, 2)).astype(bf))

    sel = np.zeros((2, P), np.float32)
    sel[0, 0:64] = 1.0
    sel[1, 64:128] = 1.0

    shared = {
        'wq': tiled(wqA), 'wk': tiled(wkA), 'wv': vfull(wvF),
        'wproj': tiled(inp['attn_proj_w'].astype(f32)),
        'wcq': tiled(wcqA), 'wck': tiled(wckA), 'wcv': vfull(wcvF),
        'wcproj': tiled(inp['cross_proj_w'].astype(f32)),
        'wfc1': tiled(wfc1),
        'wfc2': tiled(inp['fc2_w'].astype(f32)),
        'bq': _feat_major(bq), 'bk': _feat_major(bk),
        'bcq': _feat_major(bcq), 'bck': _feat_major(bck),
        'bproj': _feat_major(inp['attn_proj_b'].astype(f32)),
        'bcproj': _feat_major(inp['cross_proj_b'].astype(f32)),
        'bfc1': np.ascontiguousarray(bfc1.reshape(-1, P).T.astype(f32)),
        'bfc2': _feat_major(inp['fc2_b'].astype(f32)),
        'bv_row': bv.reshape(1, DIM).astype(bf),
        'bcv_row': bcv.reshape(1, DIM).astype(bf),
        'ones_bf': np.ones((P, P), bf),
        'rot_bf': _rot_mat().astype(bf),
        'sel_bf': sel.astype(bf),
    }
    per_core = []
    for bi in range(B):
        cxx, sxn = _rope_tables(inp['xpos'][bi])
        cyn, syn = _rope_tables(inp['ypos'][bi])
        d = {
            'xT': np.ascontiguousarray(inp['x'][bi].T.astype(f32)),
            'yT': np.ascontiguousarray(inp['y'][bi].T.astype(f32)),
            'cosx': np.ascontiguousarray(np.tile(cxx, (2, 1)).astype(bf)),
            'sinx': np.ascontiguousarray(np.tile(sxn, (2, 1)).astype(bf)),
            'cosy': np.ascontiguousarray(np.tile(cyn, (2, 1)).astype(bf)),
            'siny': np.ascontiguousarray(np.tile(syn, (2, 1)).astype(bf)),
        }
        d.update(shared)
        per_core.append(d)
    return per_core


# ------------------------------------------------------- walrus workarounds

def split_excess_waits(nc, max_waits=1):
    """This walrus build rejects instructions carrying more than one
    sync-wait on CTRL-class instructions. Move excess waits onto NoOps
    inserted immediately before the offending instruction on the same
    engine (same-engine program order keeps semantics)."""
    import concourse.mybir as mybir
    n_split = 0
    cnt = [0]
    for f in nc.m.functions:
        for blk in f.blocks:
            insts = list(blk.instructions)
            out = []
            changed = False
            for inst in insts:
                si = inst.sync_info
                waits = list(si.on_wait) if si and si.on_wait else []
                if len(waits) > max_waits:
                    changed = True
                    n_split += 1
                    extra = waits[:-max_waits]
                    keep = waits[-max_waits:]
                    while extra:
                        chunk, extra = extra[:max_waits], extra[max_waits:]
                        cnt[0] += 1
                        nop = mybir.InstNoOp(
                            name=f"WSPLIT-{id(nc) % 100000}-{cnt[0]}",
                            ins=[], outs=[], engine=inst.engine)
                        nop.sync_info = mybir.SyncInfo(on_wait=chunk,
                                                       on_update=[])
                        out.append(nop)
                    inst.sync_info = mybir.SyncInfo(
                        on_wait=keep,
                        on_update=list(si.on_update) if si.on_update else [])
                out.append(inst)
            if changed:
                blk.instructions = out
    return n_split


# ------------------------------------------------------------- kernel build

def build_nc(k_iters=1):
    import concourse.bass as bass
    import concourse.mybir as mybir
    from concourse.tile import TileContext

    F32 = mybir.dt.float32
    BF16 = mybir.dt.bfloat16
    AF = mybir.ActivationFunctionType
    OP = mybir.AluOpType

    nc = bass.Bass()
    d = {}
    for name, shape, dt in [
        ('xT', [DIM, N], F32), ('yT', [DIM, N], F32),
        ('cosx', [P, N], BF16), ('sinx', [P, N], BF16),
        ('cosy', [P, N], BF16), ('siny', [P, N], BF16),
        ('wq', [C, P, C, P], BF16), ('wk', [C, P, C, P], BF16),
        ('wv', [P, C, DIM], BF16), ('wproj', [C, P, C, P], BF16),
        ('wcq', [C, P, C, P], BF16), ('wck', [C, P, C, P], BF16),
        ('wcv', [P, C, DIM], BF16), ('wcproj', [C, P, C, P], BF16),
        ('wfc1', [HM, P, C, P], BF16), ('wfc2', [C, P, HM, P], BF16),
        ('bq', [P, C], F32), ('bk', [P, C], F32),
        ('bcq', [P, C], F32), ('bck', [P, C], F32),
        ('bproj', [P, C], F32), ('bcproj', [P, C], F32),
        ('bfc1', [P, HM], F32), ('bfc2', [P, C], F32),
        ('bv_row', [1, DIM], BF16), ('bcv_row', [1, DIM], BF16),
        ('ones_bf', [P, P], BF16), ('rot_bf', [P, P], BF16),
        ('sel_bf', [2, P], BF16),
    ]:
        d[name] = nc.declare_dram_parameter(name, shape, dt, isOutput=False)
    out_d = nc.declare_dram_parameter('outT', [DIM, N], F32, isOutput=True)

    with TileContext(nc) as tc:
        with tc.tile_pool(name="const", bufs=1) as const, \
             tc.tile_pool(name="main", bufs=1) as main, \
             tc.tile_pool(name="work", bufs=2) as work, \
             tc.tile_pool(name="ps", bufs=3, space="PSUM") as psp, \
             tc.tile_pool(name="rp", bufs=2, space="PSUM") as repp:

            def body():
                # ---- tiny constants first (so the first LN matmuls can
                # start as soon as xT chunk 0 lands) ----
                ones_bf = const.tile([P, P], BF16, tag='ones', name='ones_bf')
                nc.sync.dma_start(ones_bf[:], d['ones_bf'][:])
                rot_sb = const.tile([P, P], BF16, tag='rot', name='rot_sb')
                nc.sync.dma_start(rot_sb[:], d['rot_bf'][:])
                sel_sb = const.tile([2, P], BF16, tag='sel', name='sel_sb')
                nc.sync.dma_start(sel_sb[:], d['sel_bf'][:])

                # ---- residual + y loads (chunked) ----
                xT = main.tile([P, C, N], F32, tag='xT', name='xT')
                for cc in range(C):
                    nc.sync.dma_start(xT[:, cc, :],
                                      d['xT'][cc * P:(cc + 1) * P, :])
                yT = main.tile([P, C, N], F32, tag='big', name='yT')
                for cc in range(C):
                    nc.sync.dma_start(yT[:, cc, :],
                                      d['yT'][cc * P:(cc + 1) * P, :])

                # ---- remaining constants ----
                cos_x = const.tile([P, N], BF16, tag='cosx', name='cos_x')
                sin_x = const.tile([P, N], BF16, tag='sinx', name='sin_x')
                cos_y = const.tile([P, N], BF16, tag='cosy', name='cos_y')
                sin_y = const.tile([P, N], BF16, tag='siny', name='sin_y')
                nc.sync.dma_start(cos_x[:], d['cosx'][:])
                nc.sync.dma_start(sin_x[:], d['sinx'][:])
                nc.sync.dma_start(cos_y[:], d['cosy'][:])
                nc.sync.dma_start(sin_y[:], d['siny'][:])
                bias = {}
                for nm in ('bq', 'bk', 'bcq', 'bck', 'bproj', 'bcproj',
                           'bfc2'):
                    bias[nm] = const.tile([P, C], F32, tag=nm, name=nm)
                    nc.sync.dma_start(bias[nm][:], d[nm][:])
                bias['bfc1'] = const.tile([P, HM], F32, tag='bfc1',
                                          name='bfc1')
                nc.sync.dma_start(bias['bfc1'][:], d['bfc1'][:])
                eps_t = const.tile([P, 1], F32, tag='eps', name='eps_t')
                nc.vector.memset(eps_t[:], EPS)
                bvrep = const.tile([P, DIM], BF16, tag='bvrep', name='bvrep')
                nc.sync.dma_start(bvrep[:],
                                  d['bv_row'][:].to_broadcast((P, DIM)))
                bcvrep = const.tile([P, DIM], BF16, tag='bcvrep', name='bcvrep')
                nc.sync.dma_start(bcvrep[:],
                                  d['bcv_row'][:].to_broadcast((P, DIM)))

                def w_cols(wd, colb, r0=0, rcnt=None):
                    """Pre-tiled weight block [128, rcnt, 128] bf16."""
                    cr = wd.shape[2]
                    if rcnt is None:
                        rcnt = cr
                    t = work.tile([P, rcnt, P], BF16, tag='wlhs', bufs=3,
                                  name='w_wlhs')
                    nc.sync.dma_start(t[:], wd[colb, :, r0:r0 + rcnt, :])
                    return t

                # ---------------- layernorm (split phases) ----------------
                def ln_stats_chunk(st, src_cc, cc):
                    """Accumulate sum / sumsq matmuls for one 128-chunk."""
                    if cc == 0:
                        st['ps1'] = psp.tile([P, N], F32, tag='ps',
                                             name='ln_ps1')
                        st['ps2'] = psp.tile([P, N], F32, tag='ps',
                                             name='ln_ps2')
                    xbf = work.tile([P, N], BF16, tag='xbf', bufs=1,
                                    name='ln_xbf')
                    xsq = work.tile([P, N], BF16, tag='qb3', bufs=1,
                                    name='ln_xsq')
                    nc.vector.tensor_copy(xbf[:], src_cc)
                    nc.vector.tensor_tensor(xsq[:], xbf[:], xbf[:], OP.mult)
                    for qh in range(2):
                        sl = slice(qh * 512, qh * 512 + 512)
                        nc.tensor.matmul(
                            st['ps1'][:, sl], ones_bf[:], xbf[:, sl],
                            start=(cc == 0), stop=(cc == C - 1))
                        nc.tensor.matmul(
                            st['ps2'][:, sl], ones_bf[:], xsq[:, sl],
                            start=(cc == 0), stop=(cc == C - 1))

                def ln_finish(st, mtag, rtag):
                    m_rep = main.tile([P, N], F32, tag=mtag, name='m_rep')
                    nc.vector.tensor_scalar_mul(m_rep[:], st['ps1'][:],
                                                1.0 / DIM)
                    var = work.tile([P, N], F32, tag='lntmp', bufs=1,
                                    name='var')
                    nc.vector.tensor_tensor(var[:], m_rep[:], m_rep[:],
                                            OP.mult)
                    nc.vector.scalar_tensor_tensor(
                        var[:], st['ps2'][:], 1.0 / DIM, var[:],
                        OP.mult, OP.subtract)
                    nc.scalar.activation(var[:], var[:], AF.Ln,
                                         bias=eps_t[:])
                    rstd = main.tile([P, N], F32, tag=rtag, name='rstd')
                    nc.scalar.activation(rstd[:], var[:], AF.Exp,
                                         scale=-0.5)
                    return m_rep, rstd

                def ln_norm(src, m_rep, rstd, dst_tag):
                    h = main.tile([P, C, N], BF16, tag=dst_tag, name='h_out')
                    for cc in range(C):
                        tmpf = work.tile([P, N], F32, tag='lntmp', bufs=1,
                                         name='lntmp')
                        nc.vector.tensor_tensor(
                            tmpf[:], src[:, cc, :], m_rep[:], OP.subtract)
                        nc.vector.tensor_tensor(
                            h[:, cc, :], tmpf[:], rstd[:], OP.mult)
                    return h

                def layernorm(src, dst_tag, mtag='s_m', rtag='s_r'):
                    st = {}
                    for cc in range(C):
                        ln_stats_chunk(st, src[:, cc, :], cc)
                    m_rep, rstd = ln_finish(st, mtag, rtag)
                    return ln_norm(src, m_rep, rstd, dst_tag)

                # ---------------- V (+ ones column) build ----------------
                def build_vt(h, wv_d, bvr, tag):
                    """V+ones lhsT tile [128, T, HEADS, 128] bf16."""
                    vt = main.tile([P, T, HEADS, P], BF16, tag=tag,
                                   name='vt_' + tag)
                    nc.gpsimd.memset(vt[:], 0.0)
                    nc.gpsimd.memset(vt[:, :, 0:HEADS:2, 64:65], 1.0)
                    nc.gpsimd.memset(vt[:, :, 1:HEADS:2, 63:64], 1.0)
                    wvt = main.tile([P, C, DIM], BF16, tag='wv_full',
                                    name='wvt')
                    nc.sync.dma_start(wvt[:], wv_d[:])

                    def chunk(tci):
                        pv = psp.tile([P, N], F32, tag='ps', name='pv')
                        for cc in range(C):
                            lhs = h[:, cc, tci * P:(tci + 1) * P]
                            nc.tensor.matmul(
                                pv[:, 0:512], lhs, wvt[:, cc, 0:512],
                                start=(cc == 0), stop=(cc == C - 1))
                            nc.tensor.matmul(
                                pv[:, 512:768], lhs, wvt[:, cc, 512:768],
                                start=(cc == 0), stop=(cc == C - 1))
                        pv_h = pv[:, 0:DIM].rearrange("p (h e) -> p h e",
                                                      e=HD)
                        bv_h = bvr[:].rearrange("p (h e) -> p h e", e=HD)
                        nc.vector.tensor_tensor(
                            vt[:, tci, 0:HEADS:2, 0:64],
                            pv_h[:, 0:HEADS:2, :], bv_h[:, 0:HEADS:2, :],
                            OP.add)
                        nc.vector.tensor_tensor(
                            vt[:, tci, 1:HEADS:2, 64:128],
                            pv_h[:, 1:HEADS:2, :], bv_h[:, 1:HEADS:2, :],
                            OP.add)
                    return vt, chunk

                # ---------------- roped Q/K pair-chunk --------------------
                # Split in two phases so the rot-matmul (which waits on the
                # DVE bias-add) is emitted after the NEXT projection's
                # matmuls -> no PE head-of-line stall.
                def qk_proj(h_src, w_d, b_sb, pi, tag):
                    wa = w_cols(w_d, pi)
                    pq = psp.tile([P, N], F32, tag='ps', name='pq')
                    for cc in range(C):
                        for qh in range(2):
                            sl = slice(qh * 512, qh * 512 + 512)
                            nc.tensor.matmul(
                                pq[:, sl], wa[:, cc, :], h_src[:, cc, sl],
                                start=(cc == 0), stop=(cc == C - 1))
                    qbf = work.tile([P, N], BF16, tag=tag, bufs=1,
                                    name=tag)
                    nc.vector.tensor_scalar_add(qbf[:], pq[:],
                                                b_sb[:, pi:pi + 1])
                    return qbf

                def rope_finish(qbf, cost, sint, tag, out=None):
                    if out is None:
                        out = work.tile([P, N], BF16, tag=tag,
                                        bufs=(2 if tag == 'qro' else 1),
                                        name=f'{tag}_t')
                    pr = psp.tile([P, N], F32, tag='ps', name='pr')
                    for qh in range(2):
                        sl = slice(qh * 512, qh * 512 + 512)
                        nc.tensor.matmul(pr[:, sl], rot_sb[:], qbf[:, sl],
                                         start=True, stop=True)
                    t1 = work.tile([P, N], BF16, tag='rt1', bufs=1,
                                   name='rt1')
                    nc.vector.tensor_tensor(t1[:], qbf[:], cost[:], OP.mult)
                    t2 = work.tile([P, N], F32, tag='lntmp', bufs=1,
                                   name='rt2')
                    nc.vector.tensor_tensor(t2[:], pr[:], sint[:], OP.mult)
                    nc.vector.tensor_tensor(out[:], t2[:], t1[:], OP.add)
                    return out

                # ---------------- attention core (one head pair) ----------
                # `fillers`: callables emitting independent PE work, run at
                # the points where the PE would otherwise stall waiting on
                # ScalarE exps (AV) or the softmax epilogue.
                def attn_core(qro, kro, vt, pi, oT, fillers=()):
                    fill = list(fillers) + [None, None, None]
                    for qh in range(2):
                        qsl = slice(qh * 512, qh * 512 + 512)
                        expS = main.tile([P, KC, 2, 512], BF16,
                                         tag='big', name='expS')
                        for kc in range(KC):
                            pss = psp.tile([P, N], F32, tag='ps',
                                           name='pss')
                            ksl = slice(kc * P, (kc + 1) * P)
                            nc.tensor.matmul(
                                pss[:, 0:512], kro[0:64, ksl],
                                qro[0:64, qsl], start=True, stop=True)
                            nc.tensor.matmul(
                                pss[:, 512:1024], kro[64:128, ksl],
                                qro[64:128, qsl], start=True, stop=True)
                            nc.scalar.activation(
                                expS[:, kc, :, :], pss[:],
                                AF.Exp, scale=float(HD) ** -0.5)
                        if fill[qh * 2] is not None:
                            fill[qh * 2]()
                        pav = psp.tile([P, N], F32, tag='ps', name='pav')
                        for kc in range(KC):
                            nc.tensor.matmul(
                                pav[:, 0:512], vt[:, kc, 2 * pi, :],
                                expS[:, kc, 0, :],
                                start=(kc == 0), stop=(kc == KC - 1))
                            nc.tensor.matmul(
                                pav[:, 512:1024], vt[:, kc, 2 * pi + 1, :],
                                expS[:, kc, 1, :],
                                start=(kc == 0), stop=(kc == KC - 1))
                        # denominators: row 64 (even head), row 63 (odd).
                        # Reciprocal on ScalarE (Ln -> Exp(-x), bf16 out),
                        # rows pulled column-aligned with two tiny local
                        # DMAs, partition-broadcast by a [2,128] selector
                        # matmul. No DRAM bounce.
                        tln = work.tile([P, N], F32, tag='lntmp', bufs=1,
                                        name='tln')
                        nc.scalar.activation(tln[:], pav[:], AF.Ln)
                        trec = work.tile([P, N], BF16, tag='trec', bufs=1,
                                         name='trec')
                        nc.scalar.activation(trec[:], tln[:], AF.Exp,
                                             scale=-1.0)
                        dsb = work.tile([2, 512], BF16, tag='dsb', bufs=1,
                                        name='dsb')
                        nc.sync.dma_start(dsb[0:1, :], trec[64:65, 0:512])
                        nc.sync.dma_start(dsb[1:2, :],
                                          trec[63:64, 512:1024])
                        rep = repp.tile([P, 512], F32, tag='rep',
                                        name='rep')
                        nc.tensor.matmul(rep[:], sel_sb[:], dsb[:],
                                         start=True, stop=True)
                        # PSUM -> SBUF: the verifier rejects tensor_tensor
                        # with two PSUM operands, so stage rep in SBUF.
                        repS = work.tile([P, 512], BF16, tag='repS',
                                         bufs=1, name='repS')
                        nc.vector.tensor_copy(repS[:], rep[:])
                        nc.vector.tensor_tensor(
                            oT[0:64, pi, qsl], pav[0:64, 0:512],
                            repS[0:64, :], OP.mult)
                        nc.vector.tensor_tensor(
                            oT[64:128, pi, qsl], pav[64:128, 512:1024],
                            repS[64:128, :], OP.mult)
                        if qh == 0 and fill[1] is not None:
                            fill[1]()

                def proj_residual(oT, w_d, b_sb, st=None):
                    # LN stats for chunk m-1 are emitted after chunk m's
                    # matmuls so the stats matmuls never head-of-line
                    # block the PE behind the DVE bias-add.
                    for m in range(C):
                        pp = psp.tile([P, N], F32, tag='ps', name='pp')
                        wp = w_cols(w_d, m)
                        for cc in range(C):
                            for qh in range(2):
                                sl = slice(qh * 512, qh * 512 + 512)
                                nc.tensor.matmul(
                                    pp[:, sl], wp[:, cc, :], oT[:, cc, sl],
                                    start=(cc == 0), stop=(cc == C - 1))
                        nc.vector.scalar_tensor_tensor(
                            xT[:, m, :], pp[:], b_sb[:, m:m + 1],
                            xT[:, m, :], OP.add, OP.add)
                        if st is not None and m > 0:
                            ln_stats_chunk(st, xT[:, m - 1, :], m - 1)
                    if st is not None:
                        ln_stats_chunk(st, xT[:, C - 1, :], C - 1)

                # ================= prologue: both input layernorms ========
                h1 = layernorm(xT, 'h', 's_m', 's_r')
                y_ = layernorm(yT, 'y_', 's_m', 's_r')

                # self V; cross V build is interleaved into self-attn below
                vt, vt_chunk = build_vt(h1, d['wv'], bvrep, 'vtS')
                for tci in range(T):
                    vt_chunk(tci)
                vtc, vtc_chunk = build_vt(y_, d['wcv'], bcvrep, 'vtC')
                kroC = main.tile([P, NP, N], BF16, tag='kroC', name='kroC')

                # ================= self attention =========================
                oT1 = main.tile([P, C, N], BF16, tag='oTB', name='oT1')
                for pi in range(NP):
                    qbf_q = qk_proj(h1, d['wq'], bias['bq'], pi, 'qb1')
                    qbf_k = qk_proj(h1, d['wk'], bias['bk'], pi, 'qb2')
                    qro = rope_finish(qbf_q, cos_x, sin_x, 'qro')
                    # cross-attn K/V prep as PE gap filler
                    qbf_c = qk_proj(y_, d['wck'], bias['bck'], pi, 'qb3')
                    kro = rope_finish(qbf_k, cos_x, sin_x, 'kro')
                    if pi < 4:
                        vtc_chunk(2 * pi)
                        vtc_chunk(2 * pi + 1)
                    rope_finish(qbf_c, cos_y, sin_y, None,
                                out=kroC[:, pi, :])
                    attn_core(qro, kro, vt, pi, oT1)
                st2 = {}
                proj_residual(oT1, d['wproj'], bias['bproj'], st2)
                m2, r2 = ln_finish(st2, 's_m', 's_r')
                h2 = ln_norm(xT, m2, r2, 'h')

                # ================= cross attention ========================
                # depth-2 software pipeline: projection runs two pairs
                # ahead and the rope-finish one pair ahead, so the DVE
                # combine for qro(pi+1) completes during attn_core(pi) and
                # the scores matmuls never stall.
                oT2 = main.tile([P, C, N], BF16, tag='oTB', name='oT2')
                qp0 = qk_proj(h2, d['wcq'], bias['bcq'], 0, 'qb1')
                qro_cur = rope_finish(qp0, cos_x, sin_x, 'qro')
                qbf_p = qk_proj(h2, d['wcq'], bias['bcq'], 1, 'qb2')
                for pi in range(NP):
                    qbf_n = (qk_proj(h2, d['wcq'], bias['bcq'], pi + 2,
                                     'qb1' if pi % 2 == 0 else 'qb2')
                             if pi + 2 < NP else None)
                    qro_next = (rope_finish(qbf_p, cos_x, sin_x, 'qro')
                                if pi + 1 < NP else None)
                    attn_core(qro_cur, kroC[:, pi, :], vtc, pi, oT2)
                    qro_cur, qbf_p = qro_next, qbf_n
                st3 = {}
                proj_residual(oT2, d['wcproj'], bias['bcproj'], st3)
                m3, r3 = ln_finish(st3, 's_m', 's_r')
                h3 = ln_norm(xT, m3, r3, 'h')

                # ================= MLP (two hidden halves) ================
                outacc = main.tile([P, C, N], F32, tag='vtC', name='outacc')
                for half in range(2):
                    hidh = main.tile([P, HH, N], BF16, tag='big',
                                     name='hidh')
                    for hj in range(HH):
                        hm = half * HH + hj
                        ph = psp.tile([P, N], F32, tag='ps', name='ph')
                        wf = w_cols(d['wfc1'], hm)
                        for cc in range(C):
                            for qh in range(2):
                                sl = slice(qh * 512, qh * 512 + 512)
                                nc.tensor.matmul(
                                    ph[:, sl], wf[:, cc, :], h3[:, cc, sl],
                                    start=(cc == 0), stop=(cc == C - 1))
                        nc.scalar.activation(
                            hidh[:, hj, :], ph[:], AF.Gelu,
                            bias=bias['bfc1'][:, hm:hm + 1])
                    for m in range(C):
                        po = psp.tile([P, N], F32, tag='ps', name='po')
                        wf2 = w_cols(d['wfc2'], m, half * HH, HH)
                        for kj in range(HH):
                            for qh in range(2):
                                sl = slice(qh * 512, qh * 512 + 512)
                                nc.tensor.matmul(
                                    po[:, sl], wf2[:, kj, :],
                                    hidh[:, kj, sl],
                                    start=(kj == 0), stop=(kj == HH - 1))
                        if half == 0:
                            nc.vector.tensor_scalar_add(
                                outacc[:, m, :], po[:],
                                bias['bfc2'][:, m:m + 1])
                        else:
                            nc.vector.tensor_tensor(
                                outacc[:, m, :], outacc[:, m, :], po[:],
                                OP.add)
                            nc.vector.tensor_tensor(
                                xT[:, m, :], xT[:, m, :], outacc[:, m, :],
                                OP.add)
                            nc.sync.dma_start(
                                out_d[m * P:(m + 1) * P, :], xT[:, m, :])

            if k_iters > 1:
                with tc.For_i(0, k_iters, 1):
                    body()
            else:
                body()

    split_excess_waits(nc)
    return nc


# ------------------------------------------------------------------ driver

def kernel(**inputs):
    from concourse.bass_utils import run_bass_kernel_spmd
    per_core = prep_host(inputs)
    nc = build_nc(1)
    res = run_bass_kernel_spmd(nc, per_core, core_ids=list(range(B)))
    x_out = np.stack([np.ascontiguousarray(res.results[i]['outT'].T)
                      for i in range(B)])
    y = np.asarray(inputs['y'], dtype=np.float32)
    return (x_out.astype(np.float32), y)

